# revision 23
# baseline (speedup 1.0000x reference)
"""GCN layer (h = xW -> sym-normalized scatter-add over edges -> log_softmax)
on 8 Trainium2 NeuronCores.

Sharding: nodes (rows of x / output) are sharded across the 8 cores; edges are
partitioned by destination core; W is replicated. Each core computes
g = D^-1/2 (x W) for its node slice (bf16), a compact AllGather replicates the
40-col g to every core's HBM where it is expanded to a 256B-pitch gather
table, then each core gathers g[src] rows (SWDGE dma_gather round-robined
over 4 queues) for its edges and scatter-adds them into its destination rows
with a one-hot mask matmul on the tensor engine (edges sorted by destination
block; masks built 16 chunks at a time with a broadcast is_equal on DVE).
Self-loop terms are added directly from the local g block, not gathered.
"""
import sys

sys.path.insert(0, "/opt/trn_rl_repo")

import numpy as np

N = 100000          # nodes
F = 512             # in features
C = 40              # classes
NCORES = 8
NPC = N // NCORES   # nodes per core = 12500
PB = 98             # 128-row blocks per core (12544 padded)
NPAD = PB * 128     # 12544
DG = 128            # g table row (bf16) = 256B, 40 used + 88 pad
RANGE_ROWS = 2 * NPAD   # 25088 rows per int16 index range (4 ranges)
G_CH = 24           # chunks (of 128 edges) per dma_gather call
MB = 16             # chunks per batched DVE mask build


def _cdiv(a, b):
    return (a + b - 1) // b


def _host_prep(x, edge_index, W, b):
    import ml_dtypes
    bf16 = ml_dtypes.bfloat16
    x = np.asarray(x, dtype=np.float32)
    W = np.asarray(W, dtype=np.float32)
    b = np.asarray(b, dtype=np.float32)
    src_all = np.asarray(edge_index[0], dtype=np.int64)
    dst_all = np.asarray(edge_index[1], dtype=np.int64)

    # degree includes self-loops (reference adds them); the self-loop term
    # itself is applied on-device from the local g block, not via the
    # gathered edge stream.
    deg = (np.bincount(dst_all, minlength=N) + 1).astype(np.int32)

    core = dst_all // NPC
    dl = dst_all - core * NPC
    blk = dl >> 7
    dloc = (dl & 127).astype(np.float32)

    cs = src_all // NPC
    i_in = src_all - cs * NPC
    row = cs * NPAD + (i_in & 127) * PB + (i_in >> 7)
    rng_id = row // RANGE_ROWS
    idx16 = (row - rng_id * RANGE_ROWS).astype(np.int16)

    key = (core * PB + blk) * 4 + rng_id
    order = np.argsort(key, kind="stable")
    s_idx16 = idx16[order]
    s_dloc = dloc[order]
    cnt = np.bincount(key, minlength=NCORES * PB * 4).reshape(NCORES, PB, 4)
    bounds = np.zeros(NCORES * PB * 4 + 1, dtype=np.int64)
    np.cumsum(cnt.ravel(), out=bounds[1:])

    n_chunks = np.maximum.reduce(
        [((cnt[c] + 127) >> 7) for c in range(NCORES)]
    )  # [PB, 4] shared chunk schedule
    T_r = n_chunks.sum(axis=0)            # chunks per range
    T_r_pad = ((T_r + G_CH - 1) // G_CH) * G_CH
    K_total = int(n_chunks.sum())

    in_maps = []
    for c in range(NCORES):
        # x^T slice, padded [F, NPAD], bf16
        xT = np.zeros((F, NPAD), dtype=bf16)
        xT[:, :NPC] = x[c * NPC:(c + 1) * NPC].T
        # deg tiled [128, PB]
        degc = np.ones(NPAD, dtype=np.int32)
        degc[:NPC] = deg[c * NPC:(c + 1) * NPC]
        deg_t = degc.reshape(PB, 128).T.copy()
        # per-range idx streams + dloc matrix
        streams = [np.zeros(128 * int(T_r_pad[r]), dtype=np.int16) for r in range(4)]
        dloc_all = np.full((K_total, 128), 255.0, dtype=np.float32)
        posr = [0, 0, 0, 0]
        K = 0
        for b_ in range(PB):
            for r in range(4):
                nch = int(n_chunks[b_, r])
                if nch == 0:
                    continue
                k0 = bounds[(c * PB + b_) * 4 + r]
                k1 = bounds[(c * PB + b_) * 4 + r + 1]
                m = int(k1 - k0)
                st = streams[r]
                off = 128 * posr[r]
                st[off:off + m] = s_idx16[k0:k1]
                dl_pad = dloc_all[K:K + nch].reshape(-1)
                dl_pad[:m] = s_dloc[k0:k1]
                posr[r] += nch
                K += nch
        assert K == K_total
        # wrap idx per call of G_CH*128: [(call, 16, G_CH*8)] -> [128, 8*T_r_pad]
        idx_arrs = {}
        for r in range(4):
            if T_r_pad[r] == 0:
                continue
            lin = streams[r].reshape(-1, G_CH * 128)          # [n_calls, 4096]
            wr = lin.reshape(lin.shape[0], -1, 16).transpose(0, 2, 1)  # [n_calls,16,256]
            w16 = np.concatenate(list(wr), axis=1)            # [16, n_calls*256]
            idx_arrs[f"idx{r}"] = np.tile(w16, (8, 1)).copy()
        im = {
            "xT": xT,
            "W": W.astype(bf16),
            "bvec": b.reshape(1, C).copy(),
            "deg": deg_t.copy(),
            "dloc": dloc_all.T.astype(bf16),   # [128, K_total]
            **idx_arrs,
        }
        in_maps.append(im)
    return in_maps, n_chunks, T_r_pad, K_total


def _build_program(n_chunks, T_r_pad, K_total, single_core=False, ablate=(),
                   n_reps=1):
    import concourse.bass as bass
    import concourse.tile as tile
    from concourse import bacc, mybir, library_config
    from contextlib import ExitStack

    bf = mybir.dt.bfloat16
    f32 = mybir.dt.float32
    nc = bacc.Bacc("TRN2", target_bir_lowering=False, debug=False,
                   num_devices=1 if single_core else NCORES,
                   num_swdge_queues=4)
    gfull_t = None
    if single_core:
        gfull_t = nc.dram_tensor("gfull", [NCORES * 128, PB * DG],
                                 bf, kind="ExternalInput")

    xT_t = nc.dram_tensor("xT", [F, NPAD], bf, kind="ExternalInput")
    W_t = nc.dram_tensor("W", [F, C], bf, kind="ExternalInput")
    b_t = nc.dram_tensor("bvec", [1, C], f32, kind="ExternalInput")
    deg_t = nc.dram_tensor("deg", [128, PB], mybir.dt.int32, kind="ExternalInput")
    dloc_t = nc.dram_tensor("dloc", [128, K_total], bf, kind="ExternalInput")
    idx_ts = {}
    for r in range(4):
        if T_r_pad[r] > 0:
            idx_ts[r] = nc.dram_tensor(f"idx{r}", [128, 8 * int(T_r_pad[r])],
                                       mybir.dt.int16, kind="ExternalInput")
    out_t = nc.dram_tensor("out", [128, PB * C], f32, kind="ExternalOutput")

    with tile.TileContext(nc) as tc:
        nc.gpsimd.load_library(library_config.mlp)
        for _rep in range(n_reps):
            _build_body(nc, tc, n_chunks, T_r_pad, K_total, single_core, ablate,
                        _rep, gfull_t, xT_t, W_t, b_t, deg_t, dloc_t, idx_ts,
                        out_t)

    nc.compile()
    return nc


def _build_body(nc, tc, n_chunks, T_r_pad, K_total, single_core, ablate, rep,
                gfull_t, xT_t, W_t, b_t, deg_t, dloc_t, idx_ts, out_t):
    import concourse.bass as bass
    import concourse.tile as tile
    from concourse import mybir
    from contextlib import ExitStack

    bf = mybir.dt.bfloat16
    f32 = mybir.dt.float32
    with ExitStack() as ctx:
        const = ctx.enter_context(tc.tile_pool(name=f"const{rep}", bufs=1))
        psum = ctx.enter_context(tc.tile_pool(name=f"psum{rep}", bufs=7,
                                              space="PSUM"))
        dram = ctx.enter_context(tc.tile_pool(name=f"dram{rep}", bufs=1,
                                              space="DRAM"))

        # ---- constants ----
        W_sb = const.tile([128, 4, C], bf)
        nc.sync.dma_start(W_sb[:], W_t[:].rearrange("(a p) c -> p a c", p=128))
        dinv = const.tile([128, PB], f32)
        iota_f = const.tile([128, 128], bf)
        with tc.tile_pool(name=f"setup{rep}", bufs=1) as setup:
            deg_sb = setup.tile([128, PB], mybir.dt.int32)
            nc.sync.dma_start(deg_sb[:], deg_t[:])
            degf = setup.tile([128, PB], f32)
            nc.vector.tensor_copy(degf[:], deg_sb[:])
            recip = setup.tile([128, PB], f32)
            nc.vector.reciprocal(recip[:], degf[:])
            nc.scalar.activation(dinv[:], recip[:],
                                 mybir.ActivationFunctionType.Sqrt)
            iota_i = setup.tile([128, 128], mybir.dt.int32)
            nc.gpsimd.iota(iota_i[:], [[1, 128]], channel_multiplier=0)
            nc.vector.tensor_copy(iota_f[:], iota_i[:])
        dloc_bf = const.tile([128, K_total], bf)
        nc.sync.dma_start(dloc_bf[:], dloc_t[:])
        # bias broadcast via ones-matmul
        ones1 = const.tile([1, 128], f32)
        nc.gpsimd.memset(ones1[:], 1.0)
        b_row = const.tile([1, C], f32)
        nc.sync.dma_start(b_row[:], b_t[:])
        b_ps = psum.tile([128, C], f32, space="PSUM", tag="ps")
        nc.tensor.matmul(out=b_ps[:], lhsT=ones1[:], rhs=b_row[:],
                         start=True, stop=True)
        b_bc = const.tile([128, C], f32)
        nc.vector.tensor_copy(b_bc[:], b_ps[:])

        # preload gather index streams early on the Act HWDGE ring so they
        # land during phase-1 compute instead of gating post-AllGather gathers
        idx_sbs = {}
        for r, t in idx_ts.items():
            tl_ = const.tile([128, 8 * int(T_r_pad[r])], mybir.dt.int16,
                             tag=f"idxfull{r}")
            nc.scalar.dma_start(tl_[:], t[:])
            idx_sbs[r] = tl_

        # ---- phase 1: g = dinv * (x @ W) for own rows (bf16, compact) ----
        JG = 7  # row blocks per xT stripe group (98 = 14*7)
        g_sb = const.tile([128, PB, C], bf)
        with tc.tile_pool(name=f"ph1_{rep}", bufs=1) as ph1, \
             tc.tile_pool(name=f"xt{rep}", bufs=8) as xtp:
            for jg in range(PB // JG):
                xts = []
                for kb in range(4):
                    t = xtp.tile([128, JG * 128], bf, tag="xt")
                    nc.sync.dma_start(
                        t[:], xT_t[kb * 128:(kb + 1) * 128,
                                   jg * JG * 128:(jg + 1) * JG * 128])
                    xts.append(t)
                for jl in range(JG):
                    j = jg * JG + jl
                    ps = psum.tile([128, C], f32, space="PSUM", tag="ps")
                    for kb in range(4):
                        nc.tensor.matmul(
                            out=ps[:],
                            lhsT=xts[kb][:, jl * 128:(jl + 1) * 128],
                            rhs=W_sb[:, kb, :],
                            start=(kb == 0), stop=(kb == 3))
                    nc.vector.tensor_scalar(
                        out=g_sb[:, j, :], in0=ps[:],
                        scalar1=dinv[:, j:j + 1], scalar2=None,
                        op0=mybir.AluOpType.mult)
            ag_in = dram.tile([128, PB * C], bf)
            nc.sync.dma_start(ag_in[:], g_sb[:].rearrange("p a b -> p (a b)"))

        if single_core:
            g_view = gfull_t[:].rearrange("p (a b) -> (p a) b", b=DG)
        else:
            ag_out = dram.tile([NCORES * 128, PB * C], bf,
                               addr_space="Shared")
            nc.gpsimd.collective_compute(
                "AllGather", mybir.AluOpType.bypass,
                replica_groups=[list(range(NCORES))],
                ins=[ag_in.opt()], outs=[ag_out.opt()])
            # expand compact [1024, PB*C] to 256B-pitch gather table
            # [1024*PB, DG] (pad cols are never read); split into 8 DMAs to
            # keep per-dim element counts under the 16-bit ISA field limit
            gfull = dram.tile([NCORES * 128 * PB, DG], bf)
            gfull_v = gfull[:].rearrange("(p a) b -> p a b", a=PB)
            ag_v = ag_out[:].rearrange("p (a b) -> p a b", b=C)
            for q in range(NCORES):
                eng = nc.sync if q % 2 == 0 else nc.scalar
                eng.dma_start(
                    gfull_v[q * 128:(q + 1) * 128, :, :C],
                    ag_v[q * 128:(q + 1) * 128])
            g_view = gfull[:]

        # ---- phase 2: gather + mask-matmul scatter ----
        s_sb = const.tile([128, PB, C], f32)
        SG = 7  # blocks per pipelined softmax group (98 = 14*7)
        with tc.tile_pool(name=f"gath{rep}", bufs=4) as gp, \
             tc.tile_pool(name=f"mask{rep}", bufs=4) as mp, \
             tc.tile_pool(name=f"sm{rep}", bufs=2) as smp:
            posr = [0, 0, 0, 0]
            cur = [None, None, None, None]
            dummy_g = None
            if "nogather" in ablate:
                dummy_g = const.tile([128, G_CH, DG], bf)
                nc.gpsimd.memset(dummy_g[:], 0.0)
            K = 0
            gcall = 0
            mask_t = None
            for b_ in range(PB):
                ps = psum.tile([128, C], f32, space="PSUM", tag="ps")
                tot = int(n_chunks[b_].sum())
                done = 0
                for r in range(4):
                    nch = int(n_chunks[b_, r])
                    for k in range(nch):
                        pr = posr[r]
                        if "nogather" in ablate:
                            pass
                        elif pr % G_CH == 0:
                            t_call = pr // G_CH
                            it = idx_sbs[r][:, t_call * 8 * G_CH:
                                            (t_call + 1) * 8 * G_CH]
                            gt = gp.tile([128, G_CH, DG], bf, tag=f"g{r}")
                            nc.gpsimd.dma_gather(
                                gt[:],
                                g_view[r * RANGE_ROWS:(r + 1) * RANGE_ROWS, :],
                                it, G_CH * 128, G_CH * 128, DG,
                                single_packet=False, queue_num=gcall % 4)
                            gcall += 1
                            cur[r] = gt
                        if "nomask" in ablate:
                            mask = iota_f[:]
                        else:
                            if K % MB == 0:
                                mbe = min(MB, K_total - K)
                                mask_t = mp.tile([128, MB, 128], bf,
                                                 tag="mask")
                                iota_bc = iota_f[:].unsqueeze(1).to_broadcast(
                                    [128, mbe, 128])
                                dloc_bc = dloc_bf[:, K:K + mbe].unsqueeze(
                                    2).to_broadcast([128, mbe, 128])
                                nc.vector.tensor_tensor(
                                    out=mask_t[:, :mbe, :], in0=iota_bc,
                                    in1=dloc_bc,
                                    op=mybir.AluOpType.is_equal)
                            mask = mask_t[:, K % MB, :]
                        rhs_src = (dummy_g if "nogather" in ablate
                                   else cur[r])[:, pr % G_CH, :C]
                        if "nomm" not in ablate:
                            nc.tensor.matmul(
                                out=ps[:], lhsT=mask,
                                rhs=rhs_src,
                                start=(done == 0), stop=(done == tot - 1))
                        posr[r] += 1
                        K += 1
                        done += 1
                # self-loop diagonal term: s = scatter + g_own
                gblk = g_sb[:, b_, :]
                if "nomm" in ablate or tot == 0:
                    nc.vector.tensor_copy(s_sb[:, b_, :], gblk)
                else:
                    nc.vector.tensor_tensor(out=s_sb[:, b_, :], in0=ps[:],
                                            in1=gblk,
                                            op=mybir.AluOpType.add)
                # pipelined log_softmax + store per finished 7-block group;
                # s_sb is READ-ONLY here — all work goes through private
                # tiles (t1/esb) so no in-place hazard with the scatter loop
                if (b_ + 1) % SG == 0:
                    g0 = b_ + 1 - SG
                    sl = s_sb[:, g0:b_ + 1, :]
                    t1 = smp.tile([128, SG, C], f32, tag="t1")
                    dinv_bc = dinv[:, g0:b_ + 1].unsqueeze(2).to_broadcast(
                        [128, SG, C])
                    nc.vector.tensor_tensor(out=t1[:], in0=sl, in1=dinv_bc,
                                            op=mybir.AluOpType.mult)
                    b_bc_ap = b_bc[:].unsqueeze(1).to_broadcast([128, SG, C])
                    nc.vector.tensor_tensor(out=t1[:], in0=t1[:], in1=b_bc_ap,
                                            op=mybir.AluOpType.add)
                    rmax = smp.tile([128, SG], f32, tag="rmax")
                    nc.vector.tensor_reduce(out=rmax[:], in_=t1[:],
                                            axis=mybir.AxisListType.X,
                                            op=mybir.AluOpType.max)
                    rmax_bc = rmax[:].unsqueeze(2).to_broadcast([128, SG, C])
                    nc.vector.tensor_tensor(out=t1[:], in0=t1[:], in1=rmax_bc,
                                            op=mybir.AluOpType.subtract)
                    esb = smp.tile([128, SG, C], f32, tag="esb")
                    nc.scalar.activation(esb[:], t1[:],
                                         mybir.ActivationFunctionType.Exp)
                    ssum = smp.tile([128, SG], f32, tag="ssum")
                    nc.vector.tensor_reduce(out=ssum[:], in_=esb[:],
                                            axis=mybir.AxisListType.X,
                                            op=mybir.AluOpType.add)
                    lse = smp.tile([128, SG], f32, tag="lse")
                    nc.scalar.activation(lse[:], ssum[:],
                                         mybir.ActivationFunctionType.Ln)
                    lse_bc = lse[:].unsqueeze(2).to_broadcast([128, SG, C])
                    nc.vector.tensor_tensor(out=esb[:], in0=t1[:], in1=lse_bc,
                                            op=mybir.AluOpType.subtract)
                    nc.scalar.dma_start(
                        out_t[:, g0 * C:(b_ + 1) * C],
                        esb[:].rearrange("p a b -> p (a b)"))



_CACHE = {}


def _get_program(n_chunks, T_r_pad, K_total):
    key = (n_chunks.tobytes(), tuple(int(t) for t in T_r_pad))
    if key not in _CACHE:
        _CACHE[key] = _build_program(n_chunks, T_r_pad, K_total)
    return _CACHE[key]


def kernel(x, edge_index, W, b, _trace=False):
    from concourse.bass_utils import run_bass_kernel_spmd

    in_maps, n_chunks, T_r_pad, K_total = _host_prep(x, edge_index, W, b)
    nc = _get_program(n_chunks, T_r_pad, K_total)
    res = run_bass_kernel_spmd(nc, in_maps, core_ids=list(range(NCORES)),
                               trace=_trace)
    out = np.empty((N, C), dtype=np.float32)
    for c in range(NCORES):
        o = res.results[c]["out"].reshape(128, PB, C)
        out[c * NPC:(c + 1) * NPC] = o.transpose(1, 0, 2).reshape(NPAD, C)[:NPC]
    if _trace:
        return out, res
    return out


# revision 24
# speedup vs baseline: 2.0364x; 2.0364x over previous
"""GCN layer (h = xW -> sym-normalized scatter-add over edges -> log_softmax)
on 8 Trainium2 NeuronCores.

Sharding: nodes (rows of x / output) are sharded across the 8 cores; edges are
partitioned by destination core; W is replicated. Each core computes
g = D^-1/2 (x W) for its node slice (bf16), a compact AllGather replicates the
40-col g to every core's HBM where it is expanded to a 256B-pitch gather
table, then each core gathers g[src] rows (SWDGE dma_gather round-robined
over 4 queues) for its edges and scatter-adds them into its destination rows
with a one-hot mask matmul on the tensor engine (edges sorted by destination
block; masks built 16 chunks at a time with a broadcast is_equal on DVE).
Self-loop terms are added directly from the local g block, not gathered.
"""
import sys

sys.path.insert(0, "/opt/trn_rl_repo")

import numpy as np

N = 100000          # nodes
F = 512             # in features
C = 40              # classes
NCORES = 8
NPC = N // NCORES   # nodes per core = 12500
PB = 98             # 128-row blocks per core (12544 padded)
NPAD = PB * 128     # 12544
DG = 128            # g table row (bf16) = 256B, 40 used + 88 pad
RANGE_ROWS = 2 * NPAD   # 25088 rows per int16 index range (4 ranges)
G_CH = 24           # chunks (of 128 edges) per dma_gather call
MB = 16             # chunks per batched DVE mask build


def _cdiv(a, b):
    return (a + b - 1) // b


def _host_prep(x, edge_index, W, b):
    import ml_dtypes
    bf16 = ml_dtypes.bfloat16
    x = np.asarray(x, dtype=np.float32)
    W = np.asarray(W, dtype=np.float32)
    b = np.asarray(b, dtype=np.float32)
    src_all = np.asarray(edge_index[0], dtype=np.int64)
    dst_all = np.asarray(edge_index[1], dtype=np.int64)

    # degree includes self-loops (reference adds them); the self-loop term
    # itself is applied on-device from the local g block, not via the
    # gathered edge stream.
    deg = (np.bincount(dst_all, minlength=N) + 1).astype(np.int32)

    core = dst_all // NPC
    dl = dst_all - core * NPC
    blk = dl >> 7
    dloc = (dl & 127).astype(np.float32)

    cs = src_all // NPC
    i_in = src_all - cs * NPC
    row = cs * NPAD + (i_in & 127) * PB + (i_in >> 7)
    rng_id = row // RANGE_ROWS
    idx16 = (row - rng_id * RANGE_ROWS).astype(np.int16)

    key = (core * PB + blk) * 4 + rng_id
    order = np.argsort(key, kind="stable")
    s_idx16 = idx16[order]
    s_dloc = dloc[order]
    cnt = np.bincount(key, minlength=NCORES * PB * 4).reshape(NCORES, PB, 4)
    bounds = np.zeros(NCORES * PB * 4 + 1, dtype=np.int64)
    np.cumsum(cnt.ravel(), out=bounds[1:])

    n_chunks = np.maximum.reduce(
        [((cnt[c] + 127) >> 7) for c in range(NCORES)]
    )  # [PB, 4] shared chunk schedule
    T_r = n_chunks.sum(axis=0)            # chunks per range
    T_r_pad = ((T_r + G_CH - 1) // G_CH) * G_CH
    K_total = int(n_chunks.sum())

    in_maps = []
    for c in range(NCORES):
        # x^T slice, padded [F, NPAD], bf16
        xT = np.zeros((F, NPAD), dtype=bf16)
        xT[:, :NPC] = x[c * NPC:(c + 1) * NPC].T
        # deg tiled [128, PB]
        degc = np.ones(NPAD, dtype=np.int32)
        degc[:NPC] = deg[c * NPC:(c + 1) * NPC]
        deg_t = degc.reshape(PB, 128).T.copy()
        # per-range idx streams + dloc matrix
        streams = [np.zeros(128 * int(T_r_pad[r]), dtype=np.int16) for r in range(4)]
        dloc_all = np.full((K_total, 128), 255.0, dtype=np.float32)
        posr = [0, 0, 0, 0]
        K = 0
        for b_ in range(PB):
            for r in range(4):
                nch = int(n_chunks[b_, r])
                if nch == 0:
                    continue
                k0 = bounds[(c * PB + b_) * 4 + r]
                k1 = bounds[(c * PB + b_) * 4 + r + 1]
                m = int(k1 - k0)
                st = streams[r]
                off = 128 * posr[r]
                st[off:off + m] = s_idx16[k0:k1]
                dl_pad = dloc_all[K:K + nch].reshape(-1)
                dl_pad[:m] = s_dloc[k0:k1]
                posr[r] += nch
                K += nch
        assert K == K_total
        # wrap idx per call of G_CH*128: [(call, 16, G_CH*8)] -> [128, 8*T_r_pad]
        idx_arrs = {}
        for r in range(4):
            if T_r_pad[r] == 0:
                continue
            lin = streams[r].reshape(-1, G_CH * 128)          # [n_calls, 4096]
            wr = lin.reshape(lin.shape[0], -1, 16).transpose(0, 2, 1)  # [n_calls,16,256]
            w16 = np.concatenate(list(wr), axis=1)            # [16, n_calls*256]
            idx_arrs[f"idx{r}"] = np.tile(w16, (8, 1)).copy()
        im = {
            "xT": xT,
            "W": W.astype(bf16),
            "bvec": b.reshape(1, C).copy(),
            "deg": deg_t.copy(),
            "dloc": dloc_all.T.astype(bf16),   # [128, K_total]
            **idx_arrs,
        }
        in_maps.append(im)
    return in_maps, n_chunks, T_r_pad, K_total


def _build_program(n_chunks, T_r_pad, K_total, single_core=False, ablate=(),
                   n_reps=1):
    import concourse.bass as bass
    import concourse.tile as tile
    from concourse import bacc, mybir, library_config
    from contextlib import ExitStack

    bf = mybir.dt.bfloat16
    f32 = mybir.dt.float32
    nc = bacc.Bacc("TRN2", target_bir_lowering=False, debug=False,
                   num_devices=1 if single_core else NCORES,
                   num_swdge_queues=4)
    gfull_t = None
    if single_core:
        gfull_t = nc.dram_tensor("gfull", [NCORES * 128, PB * DG],
                                 bf, kind="ExternalInput")

    xT_t = nc.dram_tensor("xT", [F, NPAD], bf, kind="ExternalInput")
    W_t = nc.dram_tensor("W", [F, C], bf, kind="ExternalInput")
    b_t = nc.dram_tensor("bvec", [1, C], f32, kind="ExternalInput")
    deg_t = nc.dram_tensor("deg", [128, PB], mybir.dt.int32, kind="ExternalInput")
    dloc_t = nc.dram_tensor("dloc", [128, K_total], bf, kind="ExternalInput")
    idx_ts = {}
    for r in range(4):
        if T_r_pad[r] > 0:
            idx_ts[r] = nc.dram_tensor(f"idx{r}", [128, 8 * int(T_r_pad[r])],
                                       mybir.dt.int16, kind="ExternalInput")
    out_t = nc.dram_tensor("out", [128, PB * C], f32, kind="ExternalOutput")

    with tile.TileContext(nc) as tc:
        nc.gpsimd.load_library(library_config.mlp)
        for _rep in range(n_reps):
            _build_body(nc, tc, n_chunks, T_r_pad, K_total, single_core, ablate,
                        _rep, gfull_t, xT_t, W_t, b_t, deg_t, dloc_t, idx_ts,
                        out_t)

    nc.compile()
    return nc


def _build_body(nc, tc, n_chunks, T_r_pad, K_total, single_core, ablate, rep,
                gfull_t, xT_t, W_t, b_t, deg_t, dloc_t, idx_ts, out_t):
    import concourse.bass as bass
    import concourse.tile as tile
    from concourse import mybir
    from contextlib import ExitStack

    bf = mybir.dt.bfloat16
    f32 = mybir.dt.float32
    with ExitStack() as ctx:
        const = ctx.enter_context(tc.tile_pool(name=f"const{rep}", bufs=1))
        psum = ctx.enter_context(tc.tile_pool(name=f"psum{rep}", bufs=7,
                                              space="PSUM"))
        dram = ctx.enter_context(tc.tile_pool(name=f"dram{rep}", bufs=1,
                                              space="DRAM"))

        # ---- constants ----
        W_sb = const.tile([128, 4, C], bf)
        nc.sync.dma_start(W_sb[:], W_t[:].rearrange("(a p) c -> p a c", p=128))
        dinv = const.tile([128, PB], f32)
        iota_f = const.tile([128, 128], bf)
        with tc.tile_pool(name=f"setup{rep}", bufs=1) as setup:
            deg_sb = setup.tile([128, PB], mybir.dt.int32)
            nc.sync.dma_start(deg_sb[:], deg_t[:])
            degf = setup.tile([128, PB], f32)
            nc.vector.tensor_copy(degf[:], deg_sb[:])
            recip = setup.tile([128, PB], f32)
            nc.vector.reciprocal(recip[:], degf[:])
            nc.scalar.activation(dinv[:], recip[:],
                                 mybir.ActivationFunctionType.Sqrt)
            iota_i = setup.tile([128, 128], mybir.dt.int32)
            nc.gpsimd.iota(iota_i[:], [[1, 128]], channel_multiplier=0)
            nc.vector.tensor_copy(iota_f[:], iota_i[:])
        dloc_bf = const.tile([128, K_total], bf)
        nc.sync.dma_start(dloc_bf[:], dloc_t[:])
        # bias broadcast via ones-matmul
        ones1 = const.tile([1, 128], f32)
        nc.gpsimd.memset(ones1[:], 1.0)
        b_row = const.tile([1, C], f32)
        nc.sync.dma_start(b_row[:], b_t[:])
        b_ps = psum.tile([128, C], f32, space="PSUM", tag="ps")
        nc.tensor.matmul(out=b_ps[:], lhsT=ones1[:], rhs=b_row[:],
                         start=True, stop=True)
        b_bc = const.tile([128, C], f32)
        nc.vector.tensor_copy(b_bc[:], b_ps[:])

        # preload gather index streams early on the Act HWDGE ring so they
        # land during phase-1 compute instead of gating post-AllGather gathers
        idx_sbs = {}
        for r, t in idx_ts.items():
            tl_ = const.tile([128, 8 * int(T_r_pad[r])], mybir.dt.int16,
                             tag=f"idxfull{r}")
            nc.scalar.dma_start(tl_[:], t[:])
            idx_sbs[r] = tl_

        # ---- phase 1: g = dinv * (x @ W) for own rows (bf16, compact) ----
        JG = 7  # row blocks per xT stripe group (98 = 14*7)
        g_sb = const.tile([128, PB, C], bf)
        with tc.tile_pool(name=f"ph1_{rep}", bufs=1) as ph1, \
             tc.tile_pool(name=f"xt{rep}", bufs=8) as xtp:
            for jg in range(PB // JG):
                xts = []
                for kb in range(4):
                    t = xtp.tile([128, JG * 128], bf, tag="xt")
                    nc.sync.dma_start(
                        t[:], xT_t[kb * 128:(kb + 1) * 128,
                                   jg * JG * 128:(jg + 1) * JG * 128])
                    xts.append(t)
                for jl in range(JG):
                    j = jg * JG + jl
                    ps = psum.tile([128, C], f32, space="PSUM", tag="ps")
                    for kb in range(4):
                        nc.tensor.matmul(
                            out=ps[:],
                            lhsT=xts[kb][:, jl * 128:(jl + 1) * 128],
                            rhs=W_sb[:, kb, :],
                            start=(kb == 0), stop=(kb == 3))
                    nc.vector.tensor_scalar(
                        out=g_sb[:, j, :], in0=ps[:],
                        scalar1=dinv[:, j:j + 1], scalar2=None,
                        op0=mybir.AluOpType.mult)
            ag_in = dram.tile([128, PB * C], bf)
            nc.sync.dma_start(ag_in[:], g_sb[:].rearrange("p a b -> p (a b)"))

        if single_core:
            g_view = gfull_t[:].rearrange("p (a b) -> (p a) b", b=DG)
        else:
            ag_out = dram.tile([NCORES * 128, PB * C], bf,
                               addr_space="Shared")
            nc.gpsimd.collective_compute(
                "AllGather", mybir.AluOpType.bypass,
                replica_groups=[list(range(NCORES))],
                ins=[ag_in.opt()], outs=[ag_out.opt()])
            # expand compact [1024, PB*C] to 256B-pitch gather table
            # [1024*PB, DG] (pad cols are never read); split into 8 DMAs to
            # keep per-dim element counts under the 16-bit ISA field limit
            gfull = dram.tile([NCORES * 128 * PB, DG], bf)
            gfull_v = gfull[:].rearrange("(p a) b -> p a b", a=PB)
            ag_v = ag_out[:].rearrange("p (a b) -> p a b", b=C)
            for q in range(NCORES):
                eng = nc.sync if q % 2 == 0 else nc.scalar
                eng.dma_start(
                    gfull_v[q * 128:(q + 1) * 128, :, :C],
                    ag_v[q * 128:(q + 1) * 128])
            g_view = gfull[:]

        # ---- phase 2: gather + mask-matmul scatter ----
        s_sb = const.tile([128, PB, C], f32)
        with tc.tile_pool(name=f"gath{rep}", bufs=4) as gp, \
             tc.tile_pool(name=f"mask{rep}", bufs=4) as mp:
            posr = [0, 0, 0, 0]
            cur = [None, None, None, None]
            dummy_g = None
            if "nogather" in ablate:
                dummy_g = const.tile([128, G_CH, DG], bf)
                nc.gpsimd.memset(dummy_g[:], 0.0)
            K = 0
            gcall = 0
            mask_t = None
            for b_ in range(PB):
                ps = psum.tile([128, C], f32, space="PSUM", tag="ps")
                tot = int(n_chunks[b_].sum())
                done = 0
                for r in range(4):
                    nch = int(n_chunks[b_, r])
                    for k in range(nch):
                        pr = posr[r]
                        if "nogather" in ablate:
                            pass
                        elif pr % G_CH == 0:
                            t_call = pr // G_CH
                            it = idx_sbs[r][:, t_call * 8 * G_CH:
                                            (t_call + 1) * 8 * G_CH]
                            gt = gp.tile([128, G_CH, DG], bf, tag=f"g{r}")
                            nc.gpsimd.dma_gather(
                                gt[:],
                                g_view[r * RANGE_ROWS:(r + 1) * RANGE_ROWS, :],
                                it, G_CH * 128, G_CH * 128, DG,
                                single_packet=False, queue_num=gcall % 4)
                            gcall += 1
                            cur[r] = gt
                        if "nomask" in ablate:
                            mask = iota_f[:]
                        else:
                            if K % MB == 0:
                                mbe = min(MB, K_total - K)
                                mask_t = mp.tile([128, MB, 128], bf,
                                                 tag="mask")
                                iota_bc = iota_f[:].unsqueeze(1).to_broadcast(
                                    [128, mbe, 128])
                                dloc_bc = dloc_bf[:, K:K + mbe].unsqueeze(
                                    2).to_broadcast([128, mbe, 128])
                                nc.vector.tensor_tensor(
                                    out=mask_t[:, :mbe, :], in0=iota_bc,
                                    in1=dloc_bc,
                                    op=mybir.AluOpType.is_equal)
                            mask = mask_t[:, K % MB, :]
                        rhs_src = (dummy_g if "nogather" in ablate
                                   else cur[r])[:, pr % G_CH, :C]
                        if "nomm" not in ablate:
                            nc.tensor.matmul(
                                out=ps[:], lhsT=mask,
                                rhs=rhs_src,
                                start=(done == 0), stop=(done == tot - 1))
                        posr[r] += 1
                        K += 1
                        done += 1
                # self-loop diagonal term: s = scatter + g_own
                gblk = g_sb[:, b_, :]
                if "nomm" in ablate or tot == 0:
                    nc.vector.tensor_copy(s_sb[:, b_, :], gblk)
                else:
                    nc.vector.tensor_tensor(out=s_sb[:, b_, :], in0=ps[:],
                                            in1=gblk,
                                            op=mybir.AluOpType.add)

        # ---- final: out = log_softmax(s * dinv + b), in place ----
        dinv_bc = dinv[:].unsqueeze(2).to_broadcast([128, PB, C])
        nc.vector.tensor_tensor(out=s_sb[:], in0=s_sb[:], in1=dinv_bc,
                                op=mybir.AluOpType.mult)
        b_bc_ap = b_bc[:].unsqueeze(1).to_broadcast([128, PB, C])
        nc.vector.tensor_tensor(out=s_sb[:], in0=s_sb[:], in1=b_bc_ap,
                                op=mybir.AluOpType.add)
        post = ctx.enter_context(tc.tile_pool(name=f"post{rep}", bufs=1))
        rmax = post.tile([128, PB], f32)
        nc.vector.tensor_reduce(out=rmax[:], in_=s_sb[:],
                                axis=mybir.AxisListType.X, op=mybir.AluOpType.max)
        rmax_bc = rmax[:].unsqueeze(2).to_broadcast([128, PB, C])
        nc.vector.tensor_tensor(out=s_sb[:], in0=s_sb[:], in1=rmax_bc,
                                op=mybir.AluOpType.subtract)
        esb = post.tile([128, PB, C], f32)
        nc.scalar.activation(esb[:], s_sb[:], mybir.ActivationFunctionType.Exp)
        ssum = post.tile([128, PB], f32)
        nc.vector.tensor_reduce(out=ssum[:], in_=esb[:],
                                axis=mybir.AxisListType.X, op=mybir.AluOpType.add)
        lse = post.tile([128, PB], f32)
        nc.scalar.activation(lse[:], ssum[:], mybir.ActivationFunctionType.Ln)
        lse_bc = lse[:].unsqueeze(2).to_broadcast([128, PB, C])
        nc.vector.tensor_tensor(out=s_sb[:], in0=s_sb[:], in1=lse_bc,
                                op=mybir.AluOpType.subtract)
        nc.sync.dma_start(out_t[:], s_sb[:].rearrange("p a b -> p (a b)"))


_CACHE = {}


def _get_program(n_chunks, T_r_pad, K_total):
    key = (n_chunks.tobytes(), tuple(int(t) for t in T_r_pad))
    if key not in _CACHE:
        _CACHE[key] = _build_program(n_chunks, T_r_pad, K_total)
    return _CACHE[key]


def kernel(x, edge_index, W, b, _trace=False):
    from concourse.bass_utils import run_bass_kernel_spmd

    in_maps, n_chunks, T_r_pad, K_total = _host_prep(x, edge_index, W, b)
    nc = _get_program(n_chunks, T_r_pad, K_total)
    res = run_bass_kernel_spmd(nc, in_maps, core_ids=list(range(NCORES)),
                               trace=_trace)
    out = np.empty((N, C), dtype=np.float32)
    for c in range(NCORES):
        o = res.results[c]["out"].reshape(128, PB, C)
        out[c * NPC:(c + 1) * NPC] = o.transpose(1, 0, 2).reshape(NPAD, C)[:NPC]
    if _trace:
        return out, res
    return out
